# revision 19
# baseline (speedup 1.0000x reference)
"""Bass/Trainium2 kernel for nn_CRF_RNN (mean-field CRF iteration).

Math (derived from the reference):
  The constant-initialized linear layers collapse the model to a scalar
  fixed-point iteration.  With
      orig0[t,n]  = 0.01 * sum_f inputs[t,n,f]
      K2[n,c]     = sum_k kernels[n,c,k]
      denom[n]    = 0.08 + 0.02 * sum_c K2[n,c]
  the output is x broadcast over the feature dim, where
      x <- (orig0 + 0.02 * (x @ K2^T)) / denom     (3 iterations, x0 = orig0)

Distribution: kernels is sharded row-wise (output-node dim) over 8 cores.
Each core builds K2^T for its 512-row slice in SBUF (DVE k-reduction +
PE transposes), computes its slice of each mean-field step with PE matmuls
(contraction over the full node dim), and an AllGather assembles the full
x vector between steps.
"""

import os
import numpy as np

# Problem constants (hardcoded per harness contract).
T, N, F, D = 32, 4096, 8, 8
NCORES = 8
A = 0.01      # feature layer constant init
B = 0.01      # linear layer constant init
RNN_NUM = 3

_CACHE = {}


def build_program(t=T, n=N, f=F, d=D, ncores=NCORES,
                  mm_bf16=True, interleave_l22=True):
    """Build + compile the SPMD Bass program (same program for all cores)."""
    import concourse.bass as bass
    import concourse.tile as tile
    from concourse import bacc, mybir
    from concourse.masks import make_identity
    from contextlib import ExitStack

    s = n // ncores            # rows of kernels owned per core
    assert s % 128 == 0 and n % 512 == 0 and t <= 32
    ni = s // 128              # 128-row n-subtiles per core
    kc_tiles = n // 128        # contraction tiles (c dim)
    cch = n // 512             # 512-wide c chunks
    dt = mybir.dt.float32
    X = mybir.AxisListType.X
    ADD = mybir.AluOpType.add

    nc = bacc.Bacc(
        "TRN2", target_bir_lowering=False, debug=False, num_devices=ncores
    )
    kern = nc.dram_tensor("kern", [s, n, d], dt, kind="ExternalInput")
    inp = nc.dram_tensor("inp", [t, s, f], dt, kind="ExternalInput")
    out = nc.dram_tensor("out", [t, s], dt, kind="ExternalOutput")

    with ExitStack() as ctx:
        tc = ctx.enter_context(tile.TileContext(nc))
        singles = ctx.enter_context(tc.tile_pool(name="singles", bufs=1))
        raws = ctx.enter_context(tc.tile_pool(name="raws", bufs=3))
        k2ps = ctx.enter_context(tc.tile_pool(name="k2ps", bufs=3))
        k2tp = ctx.enter_context(tc.tile_pool(name="k2tp", bufs=1))
        xpool = ctx.enter_context(tc.tile_pool(name="xpool", bufs=2))
        small = ctx.enter_context(tc.tile_pool(name="small", bufs=2))
        tpps = ctx.enter_context(tc.tile_pool(name="tpps", bufs=3, space="PSUM"))
        ypps = ctx.enter_context(tc.tile_pool(name="ypps", bufs=2, space="PSUM"))
        opps = ctx.enter_context(tc.tile_pool(name="opps", bufs=1, space="PSUM"))
        dram = ctx.enter_context(tc.tile_pool(name="dram", bufs=2, space="DRAM"))

        ident = singles.tile([128, 128], dt, tag="ident", name="ident")
        make_identity(nc, ident)
        dtm = mybir.dt.bfloat16 if mm_bf16 else dt
        ones_k = singles.tile([128, 1], dtm, tag="ones_k", name="ones_k")
        nc.vector.memset(ones_k, 1.0)
        ones_m = singles.tile([1, t], dt, tag="ones_m", name="ones_m")
        nc.vector.memset(ones_m, 1.0)

        # ---- local feature reduction: o_raw = sum_f inputs_d ----
        # (small DMAs ride the SWDGE/gpsimd ring so they never block the
        #  kernel-streaming HWDGE FIFO behind a collective wait)
        ind = singles.tile([t, s, f], dt, tag="ind", name="ind")
        nc.gpsimd.dma_start(out=ind, in_=inp.ap())
        o_raw = singles.tile([t, s], dt, tag="o_raw", name="o_raw")
        nc.vector.tensor_reduce(o_raw, ind, axis=X, op=ADD)

        last_xins = None

        def xslices_to_ccin(x_tn, scale, eng):
            """(t, s) t-major slice -> cl-major (128, ni, t) DRAM bounce tile.

            cl-major keeps each partition's (ni, t) block contiguous in DRAM
            (256B runs) for both the write here and the gathered reload."""
            xins = small.tile([128, ni, t], dtm, tag="xins", name="xins")
            for j in range(ni):
                tp = tpps.tile([128, t], dt, tag="tp", name="tp")
                nc.tensor.transpose(
                    tp, x_tn[:, j * 128:(j + 1) * 128], ident[:t, :t]
                )
                if scale == 1.0:
                    nc.scalar.copy(xins[:, j, :], tp)
                else:
                    nc.scalar.mul(xins[:, j, :], tp, scale)
            cc_in = dram.tile([128, ni, t], dtm, tag="cc_in", name="cc_in")
            eng.dma_start(out=cc_in, in_=xins)
            nonlocal last_xins
            last_xins = xins
            return cc_in

        def allgather_x(cc_in, eng):
            """AllGather slices -> full x^T tiles (cl, kc, t)."""
            cc_out = dram.tile(
                [ncores, 128, ni, t], dtm, tag="cc_out", name="cc_out"
            )
            nc.gpsimd.collective_compute(
                "AllGather",
                mybir.AluOpType.bypass,
                replica_groups=[list(range(ncores))],
                ins=[cc_in.opt()],
                outs=[cc_out.opt()],
            )
            x0t = xpool.tile([128, kc_tiles, t], dtm, tag="x0t", name="x0t")
            xv = x0t.rearrange("cl (r j) t -> cl r j t", r=ncores)
            cv = cc_out.rearrange("r cl j t -> cl r j t")
            h = ncores // 2
            ld1 = eng.dma_start(out=xv[:, :h], in_=cv[:, :h])
            ld2 = eng.dma_start(out=xv[:, h:], in_=cv[:, h:])
            return x0t, (ld1, ld2)

        # initial x = orig0 (scale o_raw by A on the way out)
        xcur, x0_lds = allgather_x(xslices_to_ccin(o_raw, A, nc.gpsimd), nc.gpsimd)

        # ---- heavy phase: stream kernels, reduce k, transpose into K2T ----
        k2t = [
            k2tp.tile([128, s], dtm, tag=f"k2t{kc}", name=f"k2t{kc}")
            for kc in range(kc_tiles)
        ]
        l22_ps = opps.tile([1, s], dt, tag="l22", name="l22_ps")
        gate_chunk = cch // 2
        gate_inst = None
        for j in range(cch):
            for i in range(ni):
                k2p = k2ps.tile([128, 512], dt, tag="k2p", name="k2p")
                if j == cch - 1 and i == ni - 1:
                    for jj in range(4):
                        rawp = raws.tile(
                            [128, 128, d], dt, tag="rawp", name="rawp")
                        nc.sync.dma_start(
                            out=rawp,
                            in_=kern.ap()[
                                i * 128:(i + 1) * 128,
                                j * 512 + jj * 128:j * 512 + (jj + 1) * 128, :],
                        )
                        nc.vector.tensor_reduce(
                            k2p[:, jj * 128:(jj + 1) * 128], rawp,
                            axis=X, op=ADD)
                else:
                    raw = raws.tile([128, 512, d], dt, tag="raw", name="raw")
                    nc.sync.dma_start(
                        out=raw,
                        in_=kern.ap()[
                            i * 128:(i + 1) * 128, j * 512:(j + 1) * 512, :],
                    )
                    nc.vector.tensor_reduce(k2p, raw, axis=X, op=ADD)
                for jj in range(4):
                    kc = j * 4 + jj
                    tp = tpps.tile([128, 128], dt, tag="tp", name="tp")
                    nc.tensor.transpose(
                        tp, k2p[:, jj * 128:(jj + 1) * 128], ident
                    )
                    cp = nc.scalar.copy(k2t[kc][:, i * 128:(i + 1) * 128], tp)
                    if j == gate_chunk and i == ni - 1 and jj == 3:
                        gate_inst = cp
            # rowsum-of-K2 accumulation for this chunk's K2T tiles (f32r,
            # interleaved so it is done when the last chunk lands)
            if interleave_l22:
                for jj in range(4):
                    kc = j * 4 + jj
                    nc.tensor.matmul(
                        l22_ps, lhsT=ones_k, rhs=k2t[kc],
                        start=(kc == 0), stop=(kc == kc_tiles - 1),
                    )
        if not interleave_l22:
            for kc in range(kc_tiles):
                nc.tensor.matmul(
                    l22_ps, lhsT=ones_k, rhs=k2t[kc],
                    start=(kc == 0), stop=(kc == kc_tiles - 1),
                )

        denom_row = small.tile([1, s], dt, tag="denom_row", name="denom_row")
        # denom = 0.08 + 0.02 * rowsum
        nc.scalar.activation(
            denom_row, l22_ps, mybir.ActivationFunctionType.Copy,
            bias=float(A * f), scale=float(2.0 * B),
        )
        recip_row = small.tile([1, s], dt, tag="recip_row", name="recip_row")
        recip_scr = small.tile([1, s], dt, tag="recip_scr", name="recip_scr")
        nc.vector.reciprocal_approx_accurate(recip_row, denom_row, recip_scr)
        # broadcast 1/denom across t partitions with a K=1 ones matmul
        bc_ps = opps.tile([t, s], dt, tag="bc", name="bc_ps")
        nc.tensor.matmul(bc_ps, lhsT=ones_m, rhs=recip_row, start=True, stop=True)
        s_bc = small.tile([t, s], dt, tag="s_bc", name="s_bc")
        nc.scalar.mul(s_bc, bc_ps, float(2.0 * B))      # 0.02 / denom
        ra_bc = small.tile([t, s], dt, tag="ra_bc", name="ra_bc")
        nc.scalar.mul(ra_bc, bc_ps, float(A))           # 0.01 / denom
        b_tn = small.tile([t, s], dt, tag="b_tn", name="b_tn")
        nc.vector.tensor_mul(b_tn, o_raw, ra_bc)        # orig0 / denom

        # ---- cross-core pre-sync: a 2-byte AllGather triggered at each
        # core's phase-1 end aligns all cores before the real AG1, so AG1
        # doesn't absorb per-core streaming-phase skew on the critical path.
        from concourse.tile_rust import add_dep_helper
        sync_in = dram.tile([1, 1], dtm, tag="sync_in", name="sync_in")
        ps1 = nc.gpsimd.dma_start(out=sync_in, in_=k2t[kc_tiles - 1][0:1, 0:1])
        for _ld in x0_lds:
            add_dep_helper(ps1.ins, _ld.ins, sync=True,
                           reason="pre-sync1 after AG0 x-load")
        sync_out = dram.tile([ncores, 1], dtm, tag="sync_out", name="sync_out")
        nc.gpsimd.collective_compute(
            "AllGather",
            mybir.AluOpType.bypass,
            replica_groups=[list(range(ncores))],
            ins=[sync_in.opt()],
            outs=[sync_out.opt()],
        )

        warm_ps = opps.tile([t, s], dt, tag="warm", name="warm_ps")

        # ---- mean-field iterations ----
        for it in range(RNN_NUM):
            y_ps = ypps.tile([t, s], dt, tag="y", name="y_ps")
            for kc in range(kc_tiles):
                mm = nc.tensor.matmul(
                    y_ps, lhsT=xcur[:, kc, :], rhs=k2t[kc],
                    start=(kc == 0), stop=(kc == kc_tiles - 1),
                )
                if it == 0 and kc == 0 and gate_inst is not None:
                    # keep iter-1 matmuls out of the PE stream until the
                    # stream is half done: by then AllGather-0 (which absorbs
                    # cross-core start skew) has certainly completed, so the
                    # PE FIFO never head-of-line blocks on x0t.
                    from concourse.tile_rust import add_dep_helper
                    add_dep_helper(
                        mm.ins, gate_inst.ins, sync=True,
                        reason="defer iter-1 matmuls past mid-stream",
                    )
            x_tn = small.tile([t, s], dt, tag="x_tn", name="x_tn")
            nc.vector.tensor_mul(x_tn, y_ps, s_bc)
            nc.vector.tensor_add(x_tn, x_tn, b_tn)
            if it < RNN_NUM - 1:
                cc_in = xslices_to_ccin(x_tn, 1.0, nc.sync)
                for w in range(10):
                    nc.tensor.matmul(
                        warm_ps, lhsT=last_xins[:, 0, :], rhs=k2t[0],
                        start=True, stop=True,
                    )
                xcur, _lds = allgather_x(cc_in, nc.sync)
            else:
                nc.sync.dma_start(out=out.ap(), in_=x_tn)

    nc.compile()
    return nc


def _get_program(key=(T, N, F, D, NCORES)):
    if key not in _CACHE:
        _CACHE[key] = build_program(*key)
    return _CACHE[key]


def make_in_maps(inputs_arr, kernels_arr, t=T, n=N, f=F, d=D, ncores=NCORES):
    s = n // ncores
    inputs_arr = np.ascontiguousarray(inputs_arr, dtype=np.float32)
    kernels_arr = np.ascontiguousarray(kernels_arr, dtype=np.float32)
    in_maps = []
    for c in range(ncores):
        in_maps.append({
            "kern": kernels_arr[c * s:(c + 1) * s],
            "inp": np.ascontiguousarray(inputs_arr[:, c * s:(c + 1) * s, :]),
        })
    return in_maps


def run_device(inputs_arr, kernels_arr, trace=False, tmpdir=None):
    from concourse.bass_utils import run_bass_kernel_spmd

    nc = _get_program()
    in_maps = make_in_maps(inputs_arr, kernels_arr)
    res = run_bass_kernel_spmd(
        nc, in_maps, core_ids=list(range(NCORES)), trace=trace, tmpdir=tmpdir
    )
    slices = [res.results[c]["out"] for c in range(NCORES)]
    x = np.concatenate(slices, axis=1)          # (T, N)
    out = np.broadcast_to(x[:, :, None], (T, N, F)).copy()
    return out.astype(np.float32), res


def kernel(**inputs):
    inputs_arr = np.asarray(inputs["inputs"], dtype=np.float32)
    kernels_arr = np.asarray(inputs["kernels"], dtype=np.float32)
    out, _ = run_device(inputs_arr, kernels_arr, trace=False)
    return out


# revision 20
# speedup vs baseline: 1.0107x; 1.0107x over previous
"""Bass/Trainium2 kernel for nn_CRF_RNN (mean-field CRF iteration).

Math (derived from the reference):
  The constant-initialized linear layers collapse the model to a scalar
  fixed-point iteration.  With
      orig0[t,n]  = 0.01 * sum_f inputs[t,n,f]
      K2[n,c]     = sum_k kernels[n,c,k]
      denom[n]    = 0.08 + 0.02 * sum_c K2[n,c]
  the output is x broadcast over the feature dim, where
      x <- (orig0 + 0.02 * (x @ K2^T)) / denom     (3 iterations, x0 = orig0)

Distribution: kernels is sharded row-wise (output-node dim) over 8 cores.
Each core builds K2^T for its 512-row slice in SBUF (DVE k-reduction +
PE transposes), computes its slice of each mean-field step with PE matmuls
(contraction over the full node dim), and an AllGather assembles the full
x vector between steps.
"""

import os
import numpy as np

# Problem constants (hardcoded per harness contract).
T, N, F, D = 32, 4096, 8, 8
NCORES = 8
A = 0.01      # feature layer constant init
B = 0.01      # linear layer constant init
RNN_NUM = 3

_CACHE = {}


def build_program(t=T, n=N, f=F, d=D, ncores=NCORES,
                  mm_bf16=True, interleave_l22=True):
    """Build + compile the SPMD Bass program (same program for all cores)."""
    import concourse.bass as bass
    import concourse.tile as tile
    from concourse import bacc, mybir
    from concourse.masks import make_identity
    from contextlib import ExitStack

    s = n // ncores            # rows of kernels owned per core
    assert s % 128 == 0 and n % 512 == 0 and t <= 32
    ni = s // 128              # 128-row n-subtiles per core
    kc_tiles = n // 128        # contraction tiles (c dim)
    cch = n // 512             # 512-wide c chunks
    dt = mybir.dt.float32
    X = mybir.AxisListType.X
    ADD = mybir.AluOpType.add

    nc = bacc.Bacc(
        "TRN2", target_bir_lowering=False, debug=False, num_devices=ncores
    )
    kern = nc.dram_tensor("kern", [s, n, d], dt, kind="ExternalInput")
    inp = nc.dram_tensor("inp", [t, s, f], dt, kind="ExternalInput")
    out = nc.dram_tensor("out", [t, s], dt, kind="ExternalOutput")

    with ExitStack() as ctx:
        tc = ctx.enter_context(tile.TileContext(nc))
        singles = ctx.enter_context(tc.tile_pool(name="singles", bufs=1))
        raws = ctx.enter_context(tc.tile_pool(name="raws", bufs=3))
        k2ps = ctx.enter_context(tc.tile_pool(name="k2ps", bufs=3))
        k2tp = ctx.enter_context(tc.tile_pool(name="k2tp", bufs=1))
        xpool = ctx.enter_context(tc.tile_pool(name="xpool", bufs=2))
        small = ctx.enter_context(tc.tile_pool(name="small", bufs=2))
        tpps = ctx.enter_context(tc.tile_pool(name="tpps", bufs=3, space="PSUM"))
        ypps = ctx.enter_context(tc.tile_pool(name="ypps", bufs=2, space="PSUM"))
        opps = ctx.enter_context(tc.tile_pool(name="opps", bufs=1, space="PSUM"))
        dram = ctx.enter_context(tc.tile_pool(name="dram", bufs=2, space="DRAM"))

        ident = singles.tile([128, 128], dt, tag="ident", name="ident")
        make_identity(nc, ident)
        dtm = mybir.dt.bfloat16 if mm_bf16 else dt
        ones_k = singles.tile([128, 1], dtm, tag="ones_k", name="ones_k")
        nc.vector.memset(ones_k, 1.0)
        ones_m = singles.tile([1, t], dt, tag="ones_m", name="ones_m")
        nc.vector.memset(ones_m, 1.0)

        # ---- local feature reduction: o_raw = sum_f inputs_d ----
        # (small DMAs ride the SWDGE/gpsimd ring so they never block the
        #  kernel-streaming HWDGE FIFO behind a collective wait)
        ind = singles.tile([t, s, f], dt, tag="ind", name="ind")
        nc.gpsimd.dma_start(out=ind, in_=inp.ap())
        o_raw = singles.tile([t, s], dt, tag="o_raw", name="o_raw")
        nc.vector.tensor_reduce(o_raw, ind, axis=X, op=ADD)

        last_xins = None

        def xslices_to_ccin(x_tn, scale, eng):
            """(t, s) t-major slice -> cl-major (128, ni, t) DRAM bounce tile.

            cl-major keeps each partition's (ni, t) block contiguous in DRAM
            (256B runs) for both the write here and the gathered reload."""
            xins = small.tile([128, ni, t], dtm, tag="xins", name="xins")
            for j in range(ni):
                tp = tpps.tile([128, t], dt, tag="tp", name="tp")
                nc.tensor.transpose(
                    tp, x_tn[:, j * 128:(j + 1) * 128], ident[:t, :t]
                )
                if scale == 1.0:
                    nc.scalar.copy(xins[:, j, :], tp)
                else:
                    nc.scalar.mul(xins[:, j, :], tp, scale)
            cc_in = dram.tile([128, ni, t], dtm, tag="cc_in", name="cc_in")
            eng.dma_start(out=cc_in, in_=xins)
            nonlocal last_xins
            last_xins = xins
            return cc_in

        def allgather_x(cc_in, eng):
            """AllGather slices -> full x^T tiles (cl, kc, t)."""
            cc_out = dram.tile(
                [ncores, 128, ni, t], dtm, tag="cc_out", name="cc_out"
            )
            nc.gpsimd.collective_compute(
                "AllGather",
                mybir.AluOpType.bypass,
                replica_groups=[list(range(ncores))],
                ins=[cc_in.opt()],
                outs=[cc_out.opt()],
            )
            x0t = xpool.tile([128, kc_tiles, t], dtm, tag="x0t", name="x0t")
            xv = x0t.rearrange("cl (r j) t -> cl r j t", r=ncores)
            cv = cc_out.rearrange("r cl j t -> cl r j t")
            h = ncores // 2
            ld1 = eng.dma_start(out=xv[:, :h], in_=cv[:, :h])
            ld2 = eng.dma_start(out=xv[:, h:], in_=cv[:, h:])
            return x0t, (ld1, ld2)

        # initial x = orig0 (scale o_raw by A on the way out)
        xcur, x0_lds = allgather_x(xslices_to_ccin(o_raw, A, nc.gpsimd), nc.gpsimd)

        # ---- heavy phase: stream kernels, reduce k, transpose into K2T ----
        k2t = [
            k2tp.tile([128, s], dtm, tag=f"k2t{kc}", name=f"k2t{kc}")
            for kc in range(kc_tiles)
        ]
        l22_ps = opps.tile([1, s], dt, tag="l22", name="l22_ps")
        gate_chunk = cch // 2
        gate_inst = None
        for j in range(cch):
            for i in range(ni):
                k2p = k2ps.tile([128, 512], dt, tag="k2p", name="k2p")
                raw = raws.tile([128, 512, d], dt, tag="raw", name="raw")
                nc.sync.dma_start(
                    out=raw,
                    in_=kern.ap()[
                        i * 128:(i + 1) * 128, j * 512:(j + 1) * 512, :],
                )
                nc.vector.tensor_reduce(k2p, raw, axis=X, op=ADD)
                for jj in range(4):
                    kc = j * 4 + jj
                    tp = tpps.tile([128, 128], dt, tag="tp", name="tp")
                    nc.tensor.transpose(
                        tp, k2p[:, jj * 128:(jj + 1) * 128], ident
                    )
                    cp = nc.scalar.copy(k2t[kc][:, i * 128:(i + 1) * 128], tp)
                    if j == gate_chunk and i == ni - 1 and jj == 3:
                        gate_inst = cp
            # rowsum-of-K2 accumulation for this chunk's K2T tiles (f32r,
            # interleaved so it is done when the last chunk lands)
            if interleave_l22:
                for jj in range(4):
                    kc = j * 4 + jj
                    nc.tensor.matmul(
                        l22_ps, lhsT=ones_k, rhs=k2t[kc],
                        start=(kc == 0), stop=(kc == kc_tiles - 1),
                    )
        if not interleave_l22:
            for kc in range(kc_tiles):
                nc.tensor.matmul(
                    l22_ps, lhsT=ones_k, rhs=k2t[kc],
                    start=(kc == 0), stop=(kc == kc_tiles - 1),
                )

        denom_row = small.tile([1, s], dt, tag="denom_row", name="denom_row")
        # denom = 0.08 + 0.02 * rowsum
        nc.scalar.activation(
            denom_row, l22_ps, mybir.ActivationFunctionType.Copy,
            bias=float(A * f), scale=float(2.0 * B),
        )
        recip_row = small.tile([1, s], dt, tag="recip_row", name="recip_row")
        recip_scr = small.tile([1, s], dt, tag="recip_scr", name="recip_scr")
        nc.vector.reciprocal_approx_accurate(recip_row, denom_row, recip_scr)
        # broadcast 1/denom across t partitions with a K=1 ones matmul
        bc_ps = opps.tile([t, s], dt, tag="bc", name="bc_ps")
        nc.tensor.matmul(bc_ps, lhsT=ones_m, rhs=recip_row, start=True, stop=True)
        s_bc = small.tile([t, s], dt, tag="s_bc", name="s_bc")
        nc.scalar.mul(s_bc, bc_ps, float(2.0 * B))      # 0.02 / denom
        ra_bc = small.tile([t, s], dt, tag="ra_bc", name="ra_bc")
        nc.scalar.mul(ra_bc, bc_ps, float(A))           # 0.01 / denom
        b_tn = small.tile([t, s], dt, tag="b_tn", name="b_tn")
        nc.vector.tensor_mul(b_tn, o_raw, ra_bc)        # orig0 / denom

        # ---- cross-core pre-sync: a 2-byte AllGather triggered at each
        # core's phase-1 end aligns all cores before the real AG1, so AG1
        # doesn't absorb per-core streaming-phase skew on the critical path.
        from concourse.tile_rust import add_dep_helper
        sync_in = dram.tile([1, 1], dtm, tag="sync_in", name="sync_in")
        ps1 = nc.gpsimd.dma_start(out=sync_in, in_=k2t[kc_tiles - 1][0:1, 0:1])
        for _ld in x0_lds:
            add_dep_helper(ps1.ins, _ld.ins, sync=True,
                           reason="pre-sync1 after AG0 x-load")
        sync_out = dram.tile([ncores, 1], dtm, tag="sync_out", name="sync_out")
        nc.gpsimd.collective_compute(
            "AllGather",
            mybir.AluOpType.bypass,
            replica_groups=[list(range(ncores))],
            ins=[sync_in.opt()],
            outs=[sync_out.opt()],
        )

        warm_ps = opps.tile([t, s], dt, tag="warm", name="warm_ps")

        # ---- mean-field iterations ----
        for it in range(RNN_NUM):
            y_ps = ypps.tile([t, s], dt, tag="y", name="y_ps")
            for kc in range(kc_tiles):
                mm = nc.tensor.matmul(
                    y_ps, lhsT=xcur[:, kc, :], rhs=k2t[kc],
                    start=(kc == 0), stop=(kc == kc_tiles - 1),
                )
                if it == 0 and kc == 0 and gate_inst is not None:
                    # keep iter-1 matmuls out of the PE stream until the
                    # stream is half done: by then AllGather-0 (which absorbs
                    # cross-core start skew) has certainly completed, so the
                    # PE FIFO never head-of-line blocks on x0t.
                    from concourse.tile_rust import add_dep_helper
                    add_dep_helper(
                        mm.ins, gate_inst.ins, sync=True,
                        reason="defer iter-1 matmuls past mid-stream",
                    )
            x_tn = small.tile([t, s], dt, tag="x_tn", name="x_tn")
            nc.vector.tensor_mul(x_tn, y_ps, s_bc)
            nc.vector.tensor_add(x_tn, x_tn, b_tn)
            if it < RNN_NUM - 1:
                cc_in = xslices_to_ccin(x_tn, 1.0, nc.sync)
                for w in range(10):
                    nc.tensor.matmul(
                        warm_ps, lhsT=last_xins[:, 0, :], rhs=k2t[0],
                        start=True, stop=True,
                    )
                xcur, _lds = allgather_x(cc_in, nc.sync)
            else:
                nc.sync.dma_start(out=out.ap(), in_=x_tn)

    nc.compile()
    return nc


def _get_program(key=(T, N, F, D, NCORES)):
    if key not in _CACHE:
        _CACHE[key] = build_program(*key)
    return _CACHE[key]


def make_in_maps(inputs_arr, kernels_arr, t=T, n=N, f=F, d=D, ncores=NCORES):
    s = n // ncores
    inputs_arr = np.ascontiguousarray(inputs_arr, dtype=np.float32)
    kernels_arr = np.ascontiguousarray(kernels_arr, dtype=np.float32)
    in_maps = []
    for c in range(ncores):
        in_maps.append({
            "kern": kernels_arr[c * s:(c + 1) * s],
            "inp": np.ascontiguousarray(inputs_arr[:, c * s:(c + 1) * s, :]),
        })
    return in_maps


def run_device(inputs_arr, kernels_arr, trace=False, tmpdir=None):
    from concourse.bass_utils import run_bass_kernel_spmd

    nc = _get_program()
    in_maps = make_in_maps(inputs_arr, kernels_arr)
    res = run_bass_kernel_spmd(
        nc, in_maps, core_ids=list(range(NCORES)), trace=trace, tmpdir=tmpdir
    )
    slices = [res.results[c]["out"] for c in range(NCORES)]
    x = np.concatenate(slices, axis=1)          # (T, N)
    out = np.broadcast_to(x[:, :, None], (T, N, F)).copy()
    return out.astype(np.float32), res


def kernel(**inputs):
    inputs_arr = np.asarray(inputs["inputs"], dtype=np.float32)
    kernels_arr = np.asarray(inputs["kernels"], dtype=np.float32)
    out, _ = run_device(inputs_arr, kernels_arr, trace=False)
    return out
